# revision 3
# baseline (speedup 1.0000x reference)
"""AttnReadout (segment softmax attention readout) Trainium2 kernel, v2.

out[g] = sum_i softmax_within_graph(tanh(x @ W.T + b) @ query)[i] * x[i]

Strategy (8 NeuronCores, data-parallel over nodes, 16384 nodes/core):
  - gate matmul in fp16, node-major PSUM layout:
      H[node, fout] = xts_chunk^T @ W^T        (lhsT = x^T chunk, rhs = W^T)
    fp16 halves the x^T HBM stream and keeps the PE at 1 col/cycle.
  - bias via DVE (b varies along the free dim in this layout):
      Hb = H_psum + b_rep                      (one [128, 1024] op per 2 chunks)
  - tanh on ScalarE -> g (fp16)
  - score via DVE tensor_tensor_reduce:
      score[node] = sum_f g[node,f] * q_rep[node,f]
    This gives scores directly as PSUM-free per-partition columns - no PE
    matvec and no PE transposes (44us of PE time in v1).
  - e = exp(score - 30) on ScalarE, per-superblock [128, 16] batches.
  - Ew[node, g] = (iota[g] == seg_node) * e_node  on GpSimd (idle engine),
    output bf16.
  - den_row  += ones^T @ Ew                    (PE, bf16, [1, 128])
    out_acc  += Ew^T @ xn_chunk                (PE, bf16 xn, [128, 512])
  - host: accumulate per-graph partials across cores, divide.

Node indexing within a 2048-node superblock is (p, c)-permuted: chunk c,
partition p holds node 16p + c.  This makes each xn DMA descriptor a
contiguous 16KB block (16 consecutive node rows per partition).

exp shift is a global constant so per-core partials add directly; scores
live in [-60, 61] so exp(score - 30) stays in f32/bf16 range and graph-level
softmax accuracy is unaffected.  Measured end-to-end rel err ~2.4e-3
(fp16 gate quantization dominates; tolerance is 2e-2).
"""

import os

import numpy as np
import ml_dtypes

P = 128          # partitions
D = 512          # feature dim
G = 512          # num graphs
N_CORES = 8
SUP = 2048       # nodes per superblock
CPS = SUP // P   # 16 chunks per superblock
PAIRS = CPS // 2
KC = D // P      # 4 contraction chunks
SHIFT = 30.0     # exp(score - SHIFT)

# c16 (fp16 container) column offsets
OWT = 0                  # W^T packed [p, k*D + j] = WT[k*128+p, j]
OQ = OWT + KC * D        # q replicated to all partitions
OIO = OQ + D             # iota row (bf16 bitcast)
OONE = OIO + P           # ones (bf16 bitcast)
C16W = OONE + 8
# c32 (f32 container) column offsets
OB = 0                   # b replicated, tiled twice ([128, 1024])
OSG = OB + 2 * D         # per-chunk relative segment ids [128, NCH]

_CACHE = {}
LAST_RESULT = None  # BassKernelResults of the most recent kernel() call


def build_module(shard):
    """Build the Bass/Tile module for one core processing `shard` nodes."""
    import concourse.bacc as bacc
    import concourse.bass as bass  # noqa: F401
    import concourse.mybir as mybir
    import concourse.tile as tile

    f32 = mybir.dt.float32
    f16 = mybir.dt.float16
    bf16 = mybir.dt.bfloat16
    Tanh = mybir.ActivationFunctionType.Tanh
    Exp = mybir.ActivationFunctionType.Exp
    is_equal = mybir.AluOpType.is_equal
    mult = mybir.AluOpType.mult
    add = mybir.AluOpType.add

    assert shard % SUP == 0
    NS = shard // SUP            # superblocks
    NCH = shard // P             # total chunks
    C32W = OSG + NCH

    nc = bacc.Bacc("TRN2", target_bir_lowering=False, debug=False, enable_partition_id=False)

    xts = nc.dram_tensor("xts", [D, shard], f16, kind="ExternalInput").ap()
    xnb = nc.dram_tensor("xnb", [shard, D], bf16, kind="ExternalInput").ap()
    c16 = nc.dram_tensor("c16", [P, C16W], f16, kind="ExternalInput").ap()
    c32 = nc.dram_tensor("c32", [P, C32W], f32, kind="ExternalInput").ap()
    ov = nc.dram_tensor("ov", [P, D], f32, kind="ExternalOutput").ap()
    od = nc.dram_tensor("od", [1, P], f32, kind="ExternalOutput").ap()

    with tile.TileContext(nc) as tc:
        with (
            tc.tile_pool(name="cpool", bufs=1) as cpool,
            tc.tile_pool(name="xtpool", bufs=2) as xtpool,
            tc.tile_pool(name="xnpool", bufs=2) as xnpool,
            tc.tile_pool(name="hbpool", bufs=3) as hbpool,
            tc.tile_pool(name="gpool", bufs=3) as gpool,
            tc.tile_pool(name="jpool", bufs=2) as jpool,
            tc.tile_pool(name="scpool", bufs=2) as scpool,
            tc.tile_pool(name="epool", bufs=2) as epool,
            tc.tile_pool(name="ewpool", bufs=18) as ewpool,
            tc.tile_pool(name="opool", bufs=1) as opool,
            tc.tile_pool(name="hpool", bufs=2, space="PSUM") as hpool,
            tc.tile_pool(name="paccpool", bufs=1, space="PSUM") as paccpool,
        ):
            c16_sb = cpool.tile([P, C16W], f16, name="c16_sb")
            nc.sync.dma_start(out=c16_sb, in_=c16)
            c32_sb = cpool.tile([P, C32W], f32, name="c32_sb")
            nc.sync.dma_start(out=c32_sb, in_=c32)

            wtv = c16_sb[:, OWT:OWT + KC * D].rearrange("p (k m) -> p k m", k=KC)
            qv = c16_sb[:, OQ:OQ + D]
            iov = c16_sb[:, OIO:OIO + P].bitcast(bf16)
            onev = c16_sb[:, OONE:OONE + 8].bitcast(bf16)
            brv = c32_sb[:, OB:OB + 2 * D]
            sgv = c32_sb[:, OSG:OSG + NCH]

            shift_sb = cpool.tile([P, 1], f32, name="shift_sb")
            nc.vector.memset(shift_sb, -SHIFT)
            warm_sb = cpool.tile([1, 4], f32, name="warm_sb")

            # ---- long-lived PSUM accumulators ----
            out_acc = paccpool.tile([P, D], f32, name="out_acc", space="PSUM")
            den_acc = paccpool.tile([1, P], f32, name="den_acc", space="PSUM")
            warm_ps = paccpool.tile([1, 2], f32, name="warm_ps", space="PSUM")

            # ---- engine warm-ups: observe the constant DMAs once each ----
            nc.tensor.matmul(
                out=warm_ps,
                lhsT=onev[0:1, 0:1],
                rhs=onev[0:1, 0:2],
                start=True,
                stop=True,
            )
            nc.vector.tensor_copy(out=warm_sb[0:1, 0:1], in_=sgv[0:1, 0:1])
            nc.vector.tensor_copy(out=warm_sb[0:1, 1:2], in_=c16_sb[0:1, 0:1])
            nc.gpsimd.tensor_scalar(
                warm_sb[0:1, 2:3],
                iov[0:1, 0:1],
                sgv[0:1, 0:1],
                1.0,
                is_equal,
                mult,
            )

            def emit_readout(pend, lo, hi, after=None):
                xnv, ews, s = pend
                for c in range(lo, hi):
                    ci = s * CPS + c
                    # denominator row first: its weight load absorbs the
                    # gpsimd Ew wait so the big matmul only waits on xn DMA
                    dmm = nc.tensor.matmul(
                        out=den_acc,
                        lhsT=onev[:, 0:1],
                        rhs=ews[c],
                        start=(ci == 0),
                        stop=(ci == NCH - 1),
                    )
                    if after is not None and c == lo:
                        tile.add_dep_helper(
                            dmm.ins, after.ins, sync=False,
                            reason="readout batch rides behind its pair's gate",
                        )
                    nc.tensor.matmul(
                        out=out_acc,
                        lhsT=ews[c],
                        rhs=xnv[:, c, :],
                        start=(ci == 0),
                        stop=(ci == NCH - 1),
                    )

            pend = None  # (xnv, ews, s) of the unreduced superblock
            for s in range(NS):
                xts_t = xtpool.tile([P, KC * SUP], f16, name="xts_t")
                xv = xts_t.rearrange("p (k n) -> p k n", k=KC)
                if s == 0:
                    # split first superblock's load so the first gate matmul
                    # starts after ~1/4 of the transfer
                    for qtr in range(4):
                        nc.sync.dma_start(
                            out=xv[:, :, qtr * D:(qtr + 1) * D],
                            in_=xts[:, s * SUP + qtr * D:s * SUP + (qtr + 1) * D]
                            .rearrange("(k p) n -> p k n", p=P),
                        )
                else:
                    nc.sync.dma_start(
                        out=xv,
                        in_=xts[:, s * SUP:(s + 1) * SUP]
                        .rearrange("(k p) n -> p k n", p=P),
                    )
                xn_t = xnpool.tile([P, CPS * D], bf16, name="xn_t")
                xnv = xn_t.rearrange("p (c d) -> p c d", c=CPS)
                nc.sync.dma_start(
                    out=xnv,
                    in_=xnb[s * SUP:(s + 1) * SUP, :]
                    .rearrange("(p c) d -> p c d", p=P),
                )

                scol = scpool.tile([P, CPS], f32, name="scol")
                for pair in range(PAIRS):
                    h2 = hpool.tile([P, 2 * D], f32, name="h2", space="PSUM")
                    last_gate = None
                    for half in range(2):
                        c = pair * 2 + half
                        for k in range(KC):
                            last_gate = nc.tensor.matmul(
                                out=h2[:, half * D:(half + 1) * D],
                                lhsT=xv[:, k, c * P:(c + 1) * P],
                                rhs=wtv[:, k, :],
                                start=(k == 0),
                                stop=(k == KC - 1),
                            )
                    if pend is not None:
                        emit_readout(pend, pair * 2, pair * 2 + 2, after=last_gate)
                    hb2 = hbpool.tile([P, 2 * D], f32, name="hb2")
                    nc.vector.tensor_tensor(out=hb2, in0=h2, in1=brv, op=add)
                    g2 = gpool.tile([P, 2 * D], f16, name="g2")
                    nc.scalar.activation(out=g2, in_=hb2, func=Tanh)
                    for half in range(2):
                        c = pair * 2 + half
                        junk = jpool.tile([P, D], f16, name="junk")
                        # tensor_tensor_reduce hangs TRN2 hw in this stack;
                        # scalar_tensor_tensor's accum_out does the same
                        # fused multiply + free-dim row-sum
                        nc.vector.scalar_tensor_tensor(
                            out=junk,
                            in0=g2[:, half * D:(half + 1) * D],
                            scalar=1.0,
                            in1=qv,
                            op0=mult,
                            op1=mult,
                            accum_out=scol[:, c:c + 1],
                        )
                ecol = epool.tile([P, CPS], f32, name="ecol")
                nc.scalar.activation(
                    out=ecol, in_=scol, func=Exp, bias=shift_sb, scale=1.0
                )
                ews = []
                for c in range(CPS):
                    ci = s * CPS + c
                    ew = ewpool.tile([P, P], bf16, name="ew")
                    nc.gpsimd.tensor_scalar(
                        ew,
                        iov,
                        sgv[:, ci:ci + 1],
                        ecol[:, c:c + 1],
                        is_equal,
                        mult,
                    )
                    ews.append(ew)
                pend = (xnv, ews, s)

            emit_readout(pend, 0, CPS)

            ov_sb = opool.tile([P, D], f32, name="ov_sb")
            nc.vector.tensor_copy(out=ov_sb, in_=out_acc)
            od_sb = opool.tile([1, P], f32, name="od_sb")
            nc.vector.tensor_copy(out=od_sb, in_=den_acc)
            nc.sync.dma_start(out=ov, in_=ov_sb)
            nc.sync.dma_start(out=od, in_=od_sb)

    nc.compile()
    return nc


def _get_module(shard):
    if shard not in _CACHE:
        _CACHE[shard] = build_module(shard)
    return _CACHE[shard]


def pack_consts(W, b, q, nch):
    """Pack the fp16 and f32 constant tensors (seg columns filled per core)."""
    c16 = np.zeros((P, C16W), dtype=np.float16)
    wt = W.T.astype(np.float16)  # [f, fout]
    c16[:, OWT:OWT + KC * D] = (
        wt.reshape(KC, P, D).transpose(1, 0, 2).reshape(P, KC * D)
    )
    c16[:, OQ:OQ + D] = q.astype(np.float16)[None, :]
    c16u = c16.view(np.uint16)
    iota_bf = np.arange(P, dtype=np.float32).astype(ml_dtypes.bfloat16)
    c16u[:, OIO:OIO + P] = iota_bf.view(np.uint16)[None, :]
    ones_bf = np.ones(8, dtype=ml_dtypes.bfloat16)
    c16u[:, OONE:OONE + 8] = ones_bf.view(np.uint16)[None, :]

    c32 = np.zeros((P, OSG + nch), dtype=np.float32)
    c32[:, OB:OB + D] = b.astype(np.float32)[None, :]
    c32[:, OB + D:OB + 2 * D] = b.astype(np.float32)[None, :]
    return c16, c32


def pack_core(xs, seg):
    """Host-side packing of one core's shard -> kernel input dict + glo."""
    shard = xs.shape[0]
    ns = shard // SUP
    nch = shard // P
    glo = int(seg.min())
    width = int(seg.max()) - glo + 1
    assert width <= P, f"shard graph range {width} > {P} unsupported"
    rel = (seg - glo).astype(np.float32)
    # node (s, 16p + c) lives at chunk c, partition p
    xts = (
        np.ascontiguousarray(xs.T)
        .reshape(D, ns, P, CPS)
        .swapaxes(2, 3)
        .reshape(D, shard)
        .astype(np.float16)
    )
    xnb = xs.astype(ml_dtypes.bfloat16)
    segc = rel.reshape(ns, P, CPS).transpose(1, 0, 2).reshape(P, nch)
    return {
        "xts": np.ascontiguousarray(xts),
        "xnb": np.ascontiguousarray(xnb),
        "segc": np.ascontiguousarray(segc),
    }, glo


def kernel(**inputs):
    global LAST_RESULT
    from concourse import bass_utils

    x = np.ascontiguousarray(np.asarray(inputs["x"], dtype=np.float32))
    gp = np.asarray(inputs["graph_ptr"]).astype(np.int64)
    W = np.asarray(inputs["W"], dtype=np.float32)
    b = np.asarray(inputs["b"], dtype=np.float32)
    q = np.asarray(inputs["query"], dtype=np.float32)

    N = x.shape[0]
    shard = N // N_CORES
    assert N % N_CORES == 0
    nch = shard // P

    c16_base, c32_base = pack_consts(W, b, q, nch)

    in_maps = []
    glos = []
    for c in range(N_CORES):
        per, glo = pack_core(
            x[c * shard:(c + 1) * shard], gp[c * shard:(c + 1) * shard]
        )
        c32 = c32_base.copy()
        c32[:, OSG:OSG + nch] = per.pop("segc")
        per["c16"] = c16_base
        per["c32"] = c32
        in_maps.append(per)
        glos.append(glo)

    nc = _get_module(shard)
    trace = bool(int(os.environ.get("KERNEL_TRACE", "0")))
    res = bass_utils.run_bass_kernel_spmd(
        nc,
        in_maps,
        core_ids=list(range(N_CORES)),
        trace=trace,
        trace_cores=list(range(N_CORES)) if trace else None,
    )
    LAST_RESULT = res

    vec = np.zeros((G, D), dtype=np.float64)
    den = np.zeros((G,), dtype=np.float64)
    for c in range(N_CORES):
        g0 = glos[c]
        g1 = min(G, g0 + P)
        vec[g0:g1] += res.results[c]["ov"][: g1 - g0].astype(np.float64)
        den[g0:g1] += res.results[c]["od"][0, : g1 - g0].astype(np.float64)
    den = np.where(den == 0.0, 1.0, den)
    return (vec / den[:, None]).astype(np.float32)


# revision 6
# speedup vs baseline: 1.7777x; 1.7777x over previous
"""AttnReadout (segment softmax attention readout) Trainium2 kernel, v2.

out[g] = sum_i softmax_within_graph(tanh(x @ W.T + b) @ query)[i] * x[i]

Strategy (8 NeuronCores, data-parallel over nodes, 16384 nodes/core):
  - gate matmul in fp16, node-major PSUM layout:
      H[node, fout] = xts_chunk^T @ W^T        (lhsT = x^T chunk, rhs = W^T)
    fp16 halves the x^T HBM stream and keeps the PE at 1 col/cycle.
  - bias via DVE (b varies along the free dim in this layout):
      Hb = H_psum + b_rep                      (one [128, 1024] op per 2 chunks)
  - tanh on ScalarE -> g (fp16)
  - score via DVE tensor_tensor_reduce:
      score[node] = sum_f g[node,f] * q_rep[node,f]
    This gives scores directly as PSUM-free per-partition columns - no PE
    matvec and no PE transposes (44us of PE time in v1).
  - e = exp(score - 30) on ScalarE, per-superblock [128, 16] batches.
  - Ew[node, g] = mask[node, g] * e_node on ScalarE (activation Copy with
    per-partition scale), where mask = host-premade one-hot of seg (bf16,
    +4MB DMA/core).  GpSimd measures ~13x slower than DVE here and the DVE
    is already saturated by the bias + score ops, so the scatter matrix
    build rides the scalar engine.
  - den_row  += ones^T @ Ew                    (PE, bf16, [1, 128])
    out_acc  += Ew^T @ xn_chunk                (PE, bf16 xn, [128, 512])
  - host: accumulate per-graph partials across cores, divide.

Node indexing within a 2048-node superblock is (p, c)-permuted: chunk c,
partition p holds node 16p + c.  This makes each xn DMA descriptor a
contiguous 16KB block (16 consecutive node rows per partition).

exp shift is a global constant so per-core partials add directly; scores
live in [-60, 61] so exp(score - 30) stays in f32/bf16 range and graph-level
softmax accuracy is unaffected.  Measured end-to-end rel err ~2.4e-3
(fp16 gate quantization dominates; tolerance is 2e-2).
"""

import os

import numpy as np
import ml_dtypes

P = 128          # partitions
D = 512          # feature dim
G = 512          # num graphs
N_CORES = 8
SUP = 2048       # nodes per superblock
CPS = SUP // P   # 16 chunks per superblock
PAIRS = CPS // 2
KC = D // P      # 4 contraction chunks
SHIFT = 30.0     # exp(score - SHIFT)

# c16 (fp16 container) column offsets
OWT = 0                  # W^T packed [p, k*D + j] = WT[k*128+p, j]
OQ = OWT + KC * D        # q replicated to all partitions
OIO = OQ + D             # iota row (bf16 bitcast)
OONE = OIO + P           # ones (bf16 bitcast)
C16W = OONE + 8
# c32 (f32 container) column offsets
OB = 0                   # b replicated, tiled twice ([128, 1024])
OSG = OB + 2 * D         # per-chunk relative segment ids [128, NCH]

_CACHE = {}
LAST_RESULT = None  # BassKernelResults of the most recent kernel() call


def build_module(shard):
    """Build the Bass/Tile module for one core processing `shard` nodes."""
    import concourse.bacc as bacc
    import concourse.bass as bass  # noqa: F401
    import concourse.mybir as mybir
    import concourse.tile as tile

    f32 = mybir.dt.float32
    f16 = mybir.dt.float16
    bf16 = mybir.dt.bfloat16
    Tanh = mybir.ActivationFunctionType.Tanh
    Exp = mybir.ActivationFunctionType.Exp
    Copy = mybir.ActivationFunctionType.Copy
    is_equal = mybir.AluOpType.is_equal
    mult = mybir.AluOpType.mult
    add = mybir.AluOpType.add

    assert shard % SUP == 0
    NS = shard // SUP            # superblocks
    NCH = shard // P             # total chunks
    C32W = OSG + NCH

    nc = bacc.Bacc("TRN2", target_bir_lowering=False, debug=False, enable_partition_id=False)

    xts = nc.dram_tensor("xts", [D, shard], f16, kind="ExternalInput").ap()
    xnb = nc.dram_tensor("xnb", [shard, D], bf16, kind="ExternalInput").ap()
    c16 = nc.dram_tensor("c16", [P, C16W], f16, kind="ExternalInput").ap()
    c32 = nc.dram_tensor("c32", [P, C32W], f32, kind="ExternalInput").ap()
    msk = nc.dram_tensor("msk", [P, NCH * P], bf16, kind="ExternalInput").ap()
    ov = nc.dram_tensor("ov", [P, D], f32, kind="ExternalOutput").ap()
    od = nc.dram_tensor("od", [1, P], f32, kind="ExternalOutput").ap()

    with tile.TileContext(nc) as tc:
        with (
            tc.tile_pool(name="cpool", bufs=1) as cpool,
            tc.tile_pool(name="xtpool", bufs=2) as xtpool,
            tc.tile_pool(name="xnpool", bufs=2) as xnpool,
            tc.tile_pool(name="hbpool", bufs=3) as hbpool,
            tc.tile_pool(name="gpool", bufs=3) as gpool,
            tc.tile_pool(name="jpool", bufs=2) as jpool,
            tc.tile_pool(name="scpool", bufs=2) as scpool,
            tc.tile_pool(name="epool", bufs=2) as epool,
            tc.tile_pool(name="mkpool", bufs=2) as mkpool,
            tc.tile_pool(name="ewpool", bufs=18) as ewpool,
            tc.tile_pool(name="opool", bufs=1) as opool,
            tc.tile_pool(name="hpool", bufs=2, space="PSUM") as hpool,
            tc.tile_pool(name="paccpool", bufs=1, space="PSUM") as paccpool,
        ):
            c16_sb = cpool.tile([P, C16W], f16, name="c16_sb")
            nc.sync.dma_start(out=c16_sb, in_=c16)
            c32_sb = cpool.tile([P, C32W], f32, name="c32_sb")
            nc.sync.dma_start(out=c32_sb, in_=c32)

            wtv = c16_sb[:, OWT:OWT + KC * D].rearrange("p (k m) -> p k m", k=KC)
            qv = c16_sb[:, OQ:OQ + D]
            iov = c16_sb[:, OIO:OIO + P].bitcast(bf16)
            onev = c16_sb[:, OONE:OONE + 8].bitcast(bf16)
            brv = c32_sb[:, OB:OB + 2 * D]
            sgv = c32_sb[:, OSG:OSG + NCH]

            shift_sb = cpool.tile([P, 1], f32, name="shift_sb")
            nc.vector.memset(shift_sb, -SHIFT)
            warm_sb = cpool.tile([1, 4], f32, name="warm_sb")

            # ---- long-lived PSUM accumulators ----
            out_acc = paccpool.tile([P, D], f32, name="out_acc", space="PSUM")
            den_acc = paccpool.tile([1, P], f32, name="den_acc", space="PSUM")
            warm_ps = paccpool.tile([1, 2], f32, name="warm_ps", space="PSUM")

            # ---- engine warm-ups: observe the constant DMAs once each ----
            nc.tensor.matmul(
                out=warm_ps,
                lhsT=onev[0:1, 0:1],
                rhs=onev[0:1, 0:2],
                start=True,
                stop=True,
            )
            nc.vector.tensor_copy(out=warm_sb[0:1, 0:1], in_=sgv[0:1, 0:1])
            nc.vector.tensor_copy(out=warm_sb[0:1, 1:2], in_=c16_sb[0:1, 0:1])
            nc.scalar.copy(out=warm_sb[0:1, 2:3], in_=c32_sb[0:1, 0:1])
            nc.scalar.activation(
                out=warm_sb[0:1, 3:4],
                in_=c16_sb[0:1, 0:1],
                func=Copy,
                scale=1.0,
            )

            def emit_readout(pend, lo, hi, after=None):
                xnv, ews, s = pend
                for c in range(lo, hi):
                    ci = s * CPS + c
                    # denominator row first: its weight load absorbs the
                    # gpsimd Ew wait so the big matmul only waits on xn DMA
                    dmm = nc.tensor.matmul(
                        out=den_acc,
                        lhsT=onev[:, 0:1],
                        rhs=ews[c],
                        start=(ci == 0),
                        stop=(ci == NCH - 1),
                    )
                    if after is not None and c == lo:
                        tile.add_dep_helper(
                            dmm.ins, after.ins, sync=False,
                            reason="readout batch rides behind its pair's gate",
                        )
                    nc.tensor.matmul(
                        out=out_acc,
                        lhsT=ews[c],
                        rhs=xnv[:, c, :],
                        start=(ci == 0),
                        stop=(ci == NCH - 1),
                    )

            pend = None  # (xnv, ews, s) of the unreduced superblock
            for s in range(NS):
                xts_t = xtpool.tile([P, KC * SUP], f16, name="xts_t")
                xv = xts_t.rearrange("p (k n) -> p k n", k=KC)
                if s == 0:
                    # split first superblock's load so the first gate matmul
                    # starts after ~1/4 of the transfer
                    for qtr in range(4):
                        nc.sync.dma_start(
                            out=xv[:, :, qtr * D:(qtr + 1) * D],
                            in_=xts[:, s * SUP + qtr * D:s * SUP + (qtr + 1) * D]
                            .rearrange("(k p) n -> p k n", p=P),
                        )
                else:
                    nc.sync.dma_start(
                        out=xv,
                        in_=xts[:, s * SUP:(s + 1) * SUP]
                        .rearrange("(k p) n -> p k n", p=P),
                    )
                xn_t = xnpool.tile([P, CPS * D], bf16, name="xn_t")
                xnv = xn_t.rearrange("p (c d) -> p c d", c=CPS)
                nc.sync.dma_start(
                    out=xnv,
                    in_=xnb[s * SUP:(s + 1) * SUP, :]
                    .rearrange("(p c) d -> p c d", p=P),
                )
                mk_t = mkpool.tile([P, CPS * P], bf16, name="mk_t")
                mkv = mk_t.rearrange("p (c g) -> p c g", c=CPS)
                nc.sync.dma_start(
                    out=mkv,
                    in_=msk[:, s * CPS * P:(s + 1) * CPS * P]
                    .rearrange("p (c g) -> p c g", c=CPS),
                )

                scol = scpool.tile([P, CPS], f32, name="scol")
                for pair in range(PAIRS):
                    h2 = hpool.tile([P, 2 * D], f32, name="h2", space="PSUM")
                    last_gate = None
                    for half in range(2):
                        c = pair * 2 + half
                        for k in range(KC):
                            last_gate = nc.tensor.matmul(
                                out=h2[:, half * D:(half + 1) * D],
                                lhsT=xv[:, k, c * P:(c + 1) * P],
                                rhs=wtv[:, k, :],
                                start=(k == 0),
                                stop=(k == KC - 1),
                            )
                    if pend is not None:
                        emit_readout(pend, pair * 2, pair * 2 + 2, after=last_gate)
                    hb2 = hbpool.tile([P, 2 * D], f32, name="hb2")
                    nc.vector.tensor_tensor(out=hb2, in0=h2, in1=brv, op=add)
                    g2 = gpool.tile([P, 2 * D], f16, name="g2")
                    nc.scalar.activation(out=g2, in_=hb2, func=Tanh)
                    for half in range(2):
                        c = pair * 2 + half
                        junk = jpool.tile([P, D], f16, name="junk")
                        # tensor_tensor_reduce hangs TRN2 hw in this stack;
                        # scalar_tensor_tensor's accum_out does the same
                        # fused multiply + free-dim row-sum
                        nc.vector.scalar_tensor_tensor(
                            out=junk,
                            in0=g2[:, half * D:(half + 1) * D],
                            scalar=1.0,
                            in1=qv,
                            op0=mult,
                            op1=mult,
                            accum_out=scol[:, c:c + 1],
                        )
                ecol = epool.tile([P, CPS], f32, name="ecol")
                nc.scalar.activation(
                    out=ecol, in_=scol, func=Exp, bias=shift_sb, scale=1.0
                )
                ews = []
                for c in range(CPS):
                    ew = ewpool.tile([P, P], bf16, name="ew")
                    nc.scalar.activation(
                        out=ew,
                        in_=mkv[:, c, :],
                        func=Copy,
                        scale=ecol[:, c:c + 1],
                    )
                    ews.append(ew)
                pend = (xnv, ews, s)

            emit_readout(pend, 0, CPS)

            ov_sb = opool.tile([P, D], f32, name="ov_sb")
            nc.vector.tensor_copy(out=ov_sb, in_=out_acc)
            od_sb = opool.tile([1, P], f32, name="od_sb")
            nc.vector.tensor_copy(out=od_sb, in_=den_acc)
            nc.sync.dma_start(out=ov, in_=ov_sb)
            nc.sync.dma_start(out=od, in_=od_sb)

    nc.compile()
    return nc


def _get_module(shard):
    if shard not in _CACHE:
        _CACHE[shard] = build_module(shard)
    return _CACHE[shard]


def pack_consts(W, b, q, nch):
    """Pack the fp16 and f32 constant tensors (seg columns filled per core)."""
    c16 = np.zeros((P, C16W), dtype=np.float16)
    wt = W.T.astype(np.float16)  # [f, fout]
    c16[:, OWT:OWT + KC * D] = (
        wt.reshape(KC, P, D).transpose(1, 0, 2).reshape(P, KC * D)
    )
    c16[:, OQ:OQ + D] = q.astype(np.float16)[None, :]
    c16u = c16.view(np.uint16)
    iota_bf = np.arange(P, dtype=np.float32).astype(ml_dtypes.bfloat16)
    c16u[:, OIO:OIO + P] = iota_bf.view(np.uint16)[None, :]
    ones_bf = np.ones(8, dtype=ml_dtypes.bfloat16)
    c16u[:, OONE:OONE + 8] = ones_bf.view(np.uint16)[None, :]

    c32 = np.zeros((P, OSG + nch), dtype=np.float32)
    c32[:, OB:OB + D] = b.astype(np.float32)[None, :]
    c32[:, OB + D:OB + 2 * D] = b.astype(np.float32)[None, :]
    return c16, c32


def pack_core(xs, seg):
    """Host-side packing of one core's shard -> kernel input dict + glo."""
    shard = xs.shape[0]
    ns = shard // SUP
    nch = shard // P
    glo = int(seg.min())
    width = int(seg.max()) - glo + 1
    assert width <= P, f"shard graph range {width} > {P} unsupported"
    rel = (seg - glo).astype(np.float32)
    # node (s, 16p + c) lives at chunk c, partition p
    xts = (
        np.ascontiguousarray(xs.T)
        .reshape(D, ns, P, CPS)
        .swapaxes(2, 3)
        .reshape(D, shard)
        .astype(np.float16)
    )
    xnb = xs.astype(ml_dtypes.bfloat16)
    segc = rel.reshape(ns, P, CPS).transpose(1, 0, 2).reshape(P, nch)
    # one-hot scatter masks: msk[p, ci*P + g] = (segc[p, ci] == g)
    msk = (
        (segc[:, :, None] == np.arange(P, dtype=np.float32)[None, None, :])
        .astype(ml_dtypes.bfloat16)
        .reshape(P, nch * P)
    )
    return {
        "xts": np.ascontiguousarray(xts),
        "xnb": np.ascontiguousarray(xnb),
        "segc": np.ascontiguousarray(segc),
        "msk": np.ascontiguousarray(msk),
    }, glo


def kernel(**inputs):
    global LAST_RESULT
    from concourse import bass_utils

    x = np.ascontiguousarray(np.asarray(inputs["x"], dtype=np.float32))
    gp = np.asarray(inputs["graph_ptr"]).astype(np.int64)
    W = np.asarray(inputs["W"], dtype=np.float32)
    b = np.asarray(inputs["b"], dtype=np.float32)
    q = np.asarray(inputs["query"], dtype=np.float32)

    N = x.shape[0]
    shard = N // N_CORES
    assert N % N_CORES == 0
    nch = shard // P

    c16_base, c32_base = pack_consts(W, b, q, nch)

    in_maps = []
    glos = []
    for c in range(N_CORES):
        per, glo = pack_core(
            x[c * shard:(c + 1) * shard], gp[c * shard:(c + 1) * shard]
        )
        c32 = c32_base.copy()
        c32[:, OSG:OSG + nch] = per.pop("segc")
        per["c16"] = c16_base
        per["c32"] = c32
        in_maps.append(per)
        glos.append(glo)

    nc = _get_module(shard)
    trace = bool(int(os.environ.get("KERNEL_TRACE", "0")))
    res = bass_utils.run_bass_kernel_spmd(
        nc,
        in_maps,
        core_ids=list(range(N_CORES)),
        trace=trace,
        trace_cores=list(range(N_CORES)) if trace else None,
    )
    LAST_RESULT = res

    vec = np.zeros((G, D), dtype=np.float64)
    den = np.zeros((G,), dtype=np.float64)
    for c in range(N_CORES):
        g0 = glos[c]
        g1 = min(G, g0 + P)
        vec[g0:g1] += res.results[c]["ov"][: g1 - g0].astype(np.float64)
        den[g0:g1] += res.results[c]["od"][0, : g1 - g0].astype(np.float64)
    den = np.where(den == 0.0, 1.0, den)
    return (vec / den[:, None]).astype(np.float32)


# revision 7
# speedup vs baseline: 1.8632x; 1.0481x over previous
"""AttnReadout (segment softmax attention readout) Trainium2 kernel, v2.

out[g] = sum_i softmax_within_graph(tanh(x @ W.T + b) @ query)[i] * x[i]

Strategy (8 NeuronCores, data-parallel over nodes, 16384 nodes/core):
  - gate matmul in fp16, node-major PSUM layout:
      H[node, fout] = xts_chunk^T @ W^T        (lhsT = x^T chunk, rhs = W^T)
    fp16 halves the x^T HBM stream and keeps the PE at 1 col/cycle.
  - the tanh bias is folded into the gate on the host:  u = solve(W, b)
    gives (x + u) @ W.T = x @ W.T + b exactly, so xts ships x + u and the
    kernel never adds b (the readout stream xnb keeps the original x).
    u has rms ~0.19 for this problem's W, so fp16 quantization of x + u
    is as accurate as for x.
  - tanh on ScalarE directly from PSUM -> g (fp16)
  - score via DVE scalar_tensor_tensor accum_out:
      score[node] = sum_f g[node,f] * q_rep[node,f]
    This gives scores directly as per-partition columns - no PE matvec and
    no PE transposes (44us of PE time in v1).  (tensor_tensor_reduce hangs
    TRN2 hw in this stack; scalar_tensor_tensor's accum path works.)
  - e = exp(score - 30) on ScalarE, per-superblock [128, 16] batches.
  - Ew[node, g] = (iota[g] == seg_node) * e_node on DVE, output bf16.
    (GpSimd measures ~13x slower per element than DVE here - unusable.)
  - den_row  += ones^T @ Ew                    (PE, bf16, [1, 128])
    out_acc  += Ew^T @ xn_chunk                (PE, bf16 xn, [128, 512])
  - host: accumulate per-graph partials across cores, divide.

Node indexing within a 2048-node superblock is (p, c)-permuted: chunk c,
partition p holds node 16p + c.  This makes each xn DMA descriptor a
contiguous 16KB block (16 consecutive node rows per partition).

exp shift is a global constant so per-core partials add directly; scores
live in [-60, 61] so exp(score - 30) stays in f32/bf16 range and graph-level
softmax accuracy is unaffected.  Measured end-to-end rel err ~2.4e-3
(fp16 gate quantization dominates; tolerance is 2e-2).
"""

import os

import numpy as np
import ml_dtypes

P = 128          # partitions
D = 512          # feature dim
G = 512          # num graphs
N_CORES = 8
SUP = 2048       # nodes per superblock
CPS = SUP // P   # 16 chunks per superblock
PAIRS = CPS // 2
KC = D // P      # 4 contraction chunks
SHIFT = 30.0     # exp(score - SHIFT)

# c16 (fp16 container) column offsets
OWT = 0                  # W^T packed [p, k*D + j] = WT[k*128+p, j]
OQ = OWT + KC * D        # q replicated to all partitions
OIO = OQ + D             # iota row (bf16 bitcast)
OONE = OIO + P           # ones (bf16 bitcast)
C16W = OONE + 8
# c32 (f32 container) column offsets
OSG = 0                  # per-chunk relative segment ids [128, NCH]

_CACHE = {}
LAST_RESULT = None  # BassKernelResults of the most recent kernel() call


def build_module(shard):
    """Build the Bass/Tile module for one core processing `shard` nodes."""
    import concourse.bacc as bacc
    import concourse.bass as bass  # noqa: F401
    import concourse.mybir as mybir
    import concourse.tile as tile

    f32 = mybir.dt.float32
    f16 = mybir.dt.float16
    bf16 = mybir.dt.bfloat16
    Tanh = mybir.ActivationFunctionType.Tanh
    Exp = mybir.ActivationFunctionType.Exp
    Copy = mybir.ActivationFunctionType.Copy
    is_equal = mybir.AluOpType.is_equal
    mult = mybir.AluOpType.mult
    add = mybir.AluOpType.add

    assert shard % SUP == 0
    NS = shard // SUP            # superblocks
    NCH = shard // P             # total chunks
    C32W = OSG + NCH

    nc = bacc.Bacc("TRN2", target_bir_lowering=False, debug=False, enable_partition_id=False)

    xts = nc.dram_tensor("xts", [D, shard], f16, kind="ExternalInput").ap()
    xnb = nc.dram_tensor("xnb", [shard, D], bf16, kind="ExternalInput").ap()
    c16 = nc.dram_tensor("c16", [P, C16W], f16, kind="ExternalInput").ap()
    c32 = nc.dram_tensor("c32", [P, C32W], f32, kind="ExternalInput").ap()
    ov = nc.dram_tensor("ov", [P, D], f32, kind="ExternalOutput").ap()
    od = nc.dram_tensor("od", [1, P], f32, kind="ExternalOutput").ap()

    with tile.TileContext(nc) as tc:
        with (
            tc.tile_pool(name="cpool", bufs=1) as cpool,
            tc.tile_pool(name="xtpool", bufs=2) as xtpool,
            tc.tile_pool(name="xnpool", bufs=2) as xnpool,
            tc.tile_pool(name="gpool", bufs=3) as gpool,
            tc.tile_pool(name="jpool", bufs=2) as jpool,
            tc.tile_pool(name="scpool", bufs=2) as scpool,
            tc.tile_pool(name="epool", bufs=2) as epool,
            tc.tile_pool(name="ewpool", bufs=18) as ewpool,
            tc.tile_pool(name="opool", bufs=1) as opool,
            tc.tile_pool(name="hpool", bufs=2, space="PSUM") as hpool,
            tc.tile_pool(name="paccpool", bufs=1, space="PSUM") as paccpool,
        ):
            c16_sb = cpool.tile([P, C16W], f16, name="c16_sb")
            nc.sync.dma_start(out=c16_sb, in_=c16)
            c32_sb = cpool.tile([P, C32W], f32, name="c32_sb")
            nc.sync.dma_start(out=c32_sb, in_=c32)

            wtv = c16_sb[:, OWT:OWT + KC * D].rearrange("p (k m) -> p k m", k=KC)
            qv = c16_sb[:, OQ:OQ + D]
            iov = c16_sb[:, OIO:OIO + P].bitcast(bf16)
            onev = c16_sb[:, OONE:OONE + 8].bitcast(bf16)
            sgv = c32_sb[:, OSG:OSG + NCH]

            shift_sb = cpool.tile([P, 1], f32, name="shift_sb")
            nc.vector.memset(shift_sb, -SHIFT)
            warm_sb = cpool.tile([1, 4], f32, name="warm_sb")

            # ---- long-lived PSUM accumulators ----
            out_acc = paccpool.tile([P, D], f32, name="out_acc", space="PSUM")
            den_acc = paccpool.tile([1, P], f32, name="den_acc", space="PSUM")
            warm_ps = paccpool.tile([1, 2], f32, name="warm_ps", space="PSUM")

            # ---- engine warm-ups: observe the constant DMAs once each ----
            nc.tensor.matmul(
                out=warm_ps,
                lhsT=onev[0:1, 0:1],
                rhs=onev[0:1, 0:2],
                start=True,
                stop=True,
            )
            nc.vector.tensor_copy(out=warm_sb[0:1, 0:1], in_=sgv[0:1, 0:1])
            nc.vector.tensor_copy(out=warm_sb[0:1, 1:2], in_=c16_sb[0:1, 0:1])
            nc.scalar.copy(out=warm_sb[0:1, 2:3], in_=c32_sb[0:1, 0:1])
            nc.scalar.activation(
                out=warm_sb[0:1, 3:4],
                in_=c16_sb[0:1, 0:1],
                func=Copy,
                scale=1.0,
            )

            def emit_readout(pend, lo, hi, after=None):
                xnv, ews, s = pend
                for c in range(lo, hi):
                    ci = s * CPS + c
                    # denominator row first: its weight load absorbs the
                    # gpsimd Ew wait so the big matmul only waits on xn DMA
                    dmm = nc.tensor.matmul(
                        out=den_acc,
                        lhsT=onev[:, 0:1],
                        rhs=ews[c],
                        start=(ci == 0),
                        stop=(ci == NCH - 1),
                    )
                    if after is not None and c == lo:
                        tile.add_dep_helper(
                            dmm.ins, after.ins, sync=False,
                            reason="readout batch rides behind its pair's gate",
                        )
                    nc.tensor.matmul(
                        out=out_acc,
                        lhsT=ews[c],
                        rhs=xnv[:, c, :],
                        start=(ci == 0),
                        stop=(ci == NCH - 1),
                    )

            pend = None  # (xnv, ews, s) of the unreduced superblock
            for s in range(NS):
                xts_t = xtpool.tile([P, KC * SUP], f16, name="xts_t")
                xv = xts_t.rearrange("p (k n) -> p k n", k=KC)
                if s == 0:
                    # split first superblock's load so the first gate matmul
                    # starts after ~1/4 of the transfer
                    for qtr in range(4):
                        nc.sync.dma_start(
                            out=xv[:, :, qtr * D:(qtr + 1) * D],
                            in_=xts[:, s * SUP + qtr * D:s * SUP + (qtr + 1) * D]
                            .rearrange("(k p) n -> p k n", p=P),
                        )
                else:
                    nc.sync.dma_start(
                        out=xv,
                        in_=xts[:, s * SUP:(s + 1) * SUP]
                        .rearrange("(k p) n -> p k n", p=P),
                    )
                xn_t = xnpool.tile([P, CPS * D], bf16, name="xn_t")
                xnv = xn_t.rearrange("p (c d) -> p c d", c=CPS)
                nc.sync.dma_start(
                    out=xnv,
                    in_=xnb[s * SUP:(s + 1) * SUP, :]
                    .rearrange("(p c) d -> p c d", p=P),
                )

                scol = scpool.tile([P, CPS], f32, name="scol")
                for pair in range(PAIRS):
                    h2 = hpool.tile([P, 2 * D], f32, name="h2", space="PSUM")
                    last_gate = None
                    for half in range(2):
                        c = pair * 2 + half
                        for k in range(KC):
                            last_gate = nc.tensor.matmul(
                                out=h2[:, half * D:(half + 1) * D],
                                lhsT=xv[:, k, c * P:(c + 1) * P],
                                rhs=wtv[:, k, :],
                                start=(k == 0),
                                stop=(k == KC - 1),
                            )
                    if pend is not None:
                        emit_readout(pend, pair * 2, pair * 2 + 2, after=last_gate)
                    g2 = gpool.tile([P, 2 * D], f16, name="g2")
                    nc.scalar.activation(out=g2, in_=h2, func=Tanh)
                    for half in range(2):
                        c = pair * 2 + half
                        junk = jpool.tile([P, D], f16, name="junk")
                        # tensor_tensor_reduce hangs TRN2 hw in this stack;
                        # scalar_tensor_tensor's accum_out does the same
                        # fused multiply + free-dim row-sum
                        nc.vector.scalar_tensor_tensor(
                            out=junk,
                            in0=g2[:, half * D:(half + 1) * D],
                            scalar=1.0,
                            in1=qv,
                            op0=mult,
                            op1=mult,
                            accum_out=scol[:, c:c + 1],
                        )
                ecol = epool.tile([P, CPS], f32, name="ecol")
                nc.scalar.activation(
                    out=ecol, in_=scol, func=Exp, bias=shift_sb, scale=1.0
                )
                ews = []
                for c in range(CPS):
                    ci = s * CPS + c
                    ew = ewpool.tile([P, P], bf16, name="ew")
                    nc.vector.tensor_scalar(
                        ew,
                        iov,
                        sgv[:, ci:ci + 1],
                        ecol[:, c:c + 1],
                        is_equal,
                        mult,
                    )
                    ews.append(ew)
                pend = (xnv, ews, s)

            emit_readout(pend, 0, CPS)

            ov_sb = opool.tile([P, D], f32, name="ov_sb")
            nc.vector.tensor_copy(out=ov_sb, in_=out_acc)
            od_sb = opool.tile([1, P], f32, name="od_sb")
            nc.vector.tensor_copy(out=od_sb, in_=den_acc)
            nc.sync.dma_start(out=ov, in_=ov_sb)
            nc.sync.dma_start(out=od, in_=od_sb)

    nc.compile()
    return nc


def _get_module(shard):
    if shard not in _CACHE:
        _CACHE[shard] = build_module(shard)
    return _CACHE[shard]


def pack_consts(W, b, q, nch):
    """Pack the fp16 and f32 constant tensors (seg columns filled per core)."""
    c16 = np.zeros((P, C16W), dtype=np.float16)
    wt = W.T.astype(np.float16)  # [f, fout]
    c16[:, OWT:OWT + KC * D] = (
        wt.reshape(KC, P, D).transpose(1, 0, 2).reshape(P, KC * D)
    )
    c16[:, OQ:OQ + D] = q.astype(np.float16)[None, :]
    c16u = c16.view(np.uint16)
    iota_bf = np.arange(P, dtype=np.float32).astype(ml_dtypes.bfloat16)
    c16u[:, OIO:OIO + P] = iota_bf.view(np.uint16)[None, :]
    ones_bf = np.ones(8, dtype=ml_dtypes.bfloat16)
    c16u[:, OONE:OONE + 8] = ones_bf.view(np.uint16)[None, :]

    c32 = np.zeros((P, OSG + nch), dtype=np.float32)
    return c16, c32


def pack_core(xs, seg, u):
    """Host-side packing of one core's shard -> kernel input dict + glo.

    `u` = solve(W, b): the gate stream ships x + u so the matmul output is
    x @ W.T + b with no on-device bias add; the readout stream keeps x.
    """
    shard = xs.shape[0]
    ns = shard // SUP
    nch = shard // P
    glo = int(seg.min())
    width = int(seg.max()) - glo + 1
    assert width <= P, f"shard graph range {width} > {P} unsupported"
    rel = (seg - glo).astype(np.float32)
    # node (s, 16p + c) lives at chunk c, partition p
    xts = (
        np.ascontiguousarray((xs + u[None, :]).T)
        .reshape(D, ns, P, CPS)
        .swapaxes(2, 3)
        .reshape(D, shard)
        .astype(np.float16)
    )
    xnb = xs.astype(ml_dtypes.bfloat16)
    segc = rel.reshape(ns, P, CPS).transpose(1, 0, 2).reshape(P, nch)
    return {
        "xts": np.ascontiguousarray(xts),
        "xnb": np.ascontiguousarray(xnb),
        "segc": np.ascontiguousarray(segc),
    }, glo


def kernel(**inputs):
    global LAST_RESULT
    from concourse import bass_utils

    x = np.ascontiguousarray(np.asarray(inputs["x"], dtype=np.float32))
    gp = np.asarray(inputs["graph_ptr"]).astype(np.int64)
    W = np.asarray(inputs["W"], dtype=np.float32)
    b = np.asarray(inputs["b"], dtype=np.float32)
    q = np.asarray(inputs["query"], dtype=np.float32)

    N = x.shape[0]
    shard = N // N_CORES
    assert N % N_CORES == 0
    nch = shard // P

    c16_base, c32_base = pack_consts(W, b, q, nch)
    u = np.linalg.solve(W.astype(np.float64), b.astype(np.float64)).astype(
        np.float32
    )

    in_maps = []
    glos = []
    for c in range(N_CORES):
        per, glo = pack_core(
            x[c * shard:(c + 1) * shard], gp[c * shard:(c + 1) * shard], u
        )
        c32 = c32_base.copy()
        c32[:, OSG:OSG + nch] = per.pop("segc")
        per["c16"] = c16_base
        per["c32"] = c32
        in_maps.append(per)
        glos.append(glo)

    nc = _get_module(shard)
    trace = bool(int(os.environ.get("KERNEL_TRACE", "0")))
    res = bass_utils.run_bass_kernel_spmd(
        nc,
        in_maps,
        core_ids=list(range(N_CORES)),
        trace=trace,
        trace_cores=list(range(N_CORES)) if trace else None,
    )
    LAST_RESULT = res

    vec = np.zeros((G, D), dtype=np.float64)
    den = np.zeros((G,), dtype=np.float64)
    for c in range(N_CORES):
        g0 = glos[c]
        g1 = min(G, g0 + P)
        vec[g0:g1] += res.results[c]["ov"][: g1 - g0].astype(np.float64)
        den[g0:g1] += res.results[c]["od"][0, : g1 - g0].astype(np.float64)
    den = np.where(den == 0.0, 1.0, den)
    return (vec / den[:, None]).astype(np.float32)


# revision 9
# speedup vs baseline: 2.0574x; 1.1042x over previous
"""AttnReadout (segment softmax attention readout) Trainium2 kernel, v2.

out[g] = sum_i softmax_within_graph(tanh(x @ W.T + b) @ query)[i] * x[i]

Strategy (8 NeuronCores, data-parallel over nodes, 16384 nodes/core):
  - gate matmul in fp16, node-major PSUM layout:
      H[node, fout] = xts_chunk^T @ W^T        (lhsT = x^T chunk, rhs = W^T)
    fp16 halves the x^T HBM stream and keeps the PE at 1 col/cycle.
  - the tanh bias is folded into the gate on the host:  u = solve(W, b)
    gives (x + u) @ W.T = x @ W.T + b exactly, so xts ships x + u and the
    kernel never adds b (the readout stream xnb keeps the original x).
    u has rms ~0.19 for this problem's W, so fp16 quantization of x + u
    is as accurate as for x.
  - tanh on ScalarE directly from PSUM -> g (fp16)
  - score via DVE scalar_tensor_tensor accum_out:
      score[node] = sum_f g[node,f] * q_rep[node,f]
    This gives scores directly as per-partition columns - no PE matvec and
    no PE transposes (44us of PE time in v1).  (tensor_tensor_reduce hangs
    TRN2 hw in this stack; scalar_tensor_tensor's accum path works.)
  - e = exp(score - 30) on ScalarE, per-superblock [128, 16] batches.
  - Ew[node, g] = (iota[g] == seg_node) * e_node on DVE, output bf16.
    (GpSimd measures ~13x slower per element than DVE here - unusable.)
  - den_row  += ones^T @ Ew                    (PE, bf16, [1, 128])
    out_acc  += Ew^T @ xn_chunk                (PE, bf16 xn, [128, 512])
  - host: accumulate per-graph partials across cores, divide.

Node indexing within a 2048-node superblock is (p, c)-permuted: chunk c,
partition p holds node 16p + c.  This makes each xn DMA descriptor a
contiguous 16KB block (16 consecutive node rows per partition).

exp shift is a global constant so per-core partials add directly; scores
live in [-60, 61] so exp(score - 30) stays in f32/bf16 range and graph-level
softmax accuracy is unaffected.  Measured end-to-end rel err ~2.4e-3
(fp16 gate quantization dominates; tolerance is 2e-2).
"""

import os

import numpy as np
import ml_dtypes

P = 128          # partitions
D = 512          # feature dim
G = 512          # num graphs
N_CORES = 8
SUP = 2048       # nodes per superblock
CPS = SUP // P   # 16 chunks per superblock
PAIRS = CPS // 2
KC = D // P      # 4 contraction chunks
SHIFT = 30.0     # exp(score - SHIFT)

# c16 (fp16 container) column offsets
OQ = 0                   # q replicated to all partitions
OIO = OQ + D             # iota row (bf16 bitcast)
OONE = OIO + P           # ones (bf16 bitcast)
C16W = OONE + 8
# c32 (f32 container) column offsets
OSG = 0                  # per-chunk relative segment ids [128, NCH]

_CACHE = {}
LAST_RESULT = None  # BassKernelResults of the most recent kernel() call


def build_module(shard):
    """Build the Bass/Tile module for one core processing `shard` nodes."""
    import concourse.bacc as bacc
    import concourse.bass as bass  # noqa: F401
    import concourse.mybir as mybir
    import concourse.tile as tile

    f32 = mybir.dt.float32
    f16 = mybir.dt.float16
    bf16 = mybir.dt.bfloat16
    Tanh = mybir.ActivationFunctionType.Tanh
    Exp = mybir.ActivationFunctionType.Exp
    Copy = mybir.ActivationFunctionType.Copy
    is_equal = mybir.AluOpType.is_equal
    mult = mybir.AluOpType.mult
    add = mybir.AluOpType.add

    assert shard % SUP == 0
    NS = shard // SUP            # superblocks
    NCH = shard // P             # total chunks
    C32W = OSG + NCH

    nc = bacc.Bacc("TRN2", target_bir_lowering=False, debug=False, enable_partition_id=False)

    xts = nc.dram_tensor("xts", [D, shard], f16, kind="ExternalInput").ap()
    xnb = nc.dram_tensor("xnb", [shard, D], bf16, kind="ExternalInput").ap()
    wt16 = nc.dram_tensor("wt16", [P, KC * D], f16, kind="ExternalInput").ap()
    c16 = nc.dram_tensor("c16", [P, C16W], f16, kind="ExternalInput").ap()
    c32 = nc.dram_tensor("c32", [P, C32W], f32, kind="ExternalInput").ap()
    ov = nc.dram_tensor("ov", [P, D], f32, kind="ExternalOutput").ap()
    od = nc.dram_tensor("od", [1, 4 * P], f32, kind="ExternalOutput").ap()

    with tile.TileContext(nc) as tc:
        with (
            tc.tile_pool(name="cpool", bufs=1) as cpool,
            tc.tile_pool(name="xtpool", bufs=2) as xtpool,
            tc.tile_pool(name="xnpool", bufs=2) as xnpool,
            tc.tile_pool(name="gpool", bufs=3) as gpool,
            tc.tile_pool(name="jpool", bufs=2) as jpool,
            tc.tile_pool(name="scpool", bufs=2) as scpool,
            tc.tile_pool(name="epool", bufs=2) as epool,
            tc.tile_pool(name="ewpool", bufs=6) as ewpool,
            tc.tile_pool(name="opool", bufs=1) as opool,
            tc.tile_pool(name="hpool", bufs=2, space="PSUM") as hpool,
            tc.tile_pool(name="paccpool", bufs=1, space="PSUM") as paccpool,
        ):
            c16_sb = cpool.tile([P, C16W], f16, name="c16_sb")
            nc.sync.dma_start(out=c16_sb, in_=c16)
            c32_sb = cpool.tile([P, C32W], f32, name="c32_sb")
            nc.sync.dma_start(out=c32_sb, in_=c32)
            wt_sb = cpool.tile([P, KC * D], f16, name="wt_sb")
            wtv = wt_sb.rearrange("p (k m) -> p k m", k=KC)
            for k in range(KC):
                nc.sync.dma_start(
                    out=wtv[:, k, :], in_=wt16[:, k * D:(k + 1) * D]
                )

            qv = c16_sb[:, OQ:OQ + D]
            iov = c16_sb[:, OIO:OIO + P].bitcast(bf16)
            onev = c16_sb[:, OONE:OONE + 8].bitcast(bf16)
            sgv = c32_sb[:, OSG:OSG + NCH]

            shift_sb = cpool.tile([P, 1], f32, name="shift_sb")
            nc.vector.memset(shift_sb, -SHIFT)
            warm_sb = cpool.tile([1, 4], f32, name="warm_sb")

            # ---- long-lived PSUM accumulators ----
            out_acc = paccpool.tile([P, D], f32, name="out_acc", space="PSUM")
            # batched den: 4 chunk-partials side by side, host sums them
            den_acc = paccpool.tile([1, 4 * P], f32, name="den_acc", space="PSUM")
            warm_ps = paccpool.tile([1, 2], f32, name="warm_ps", space="PSUM")

            # ---- engine warm-ups: observe the constant DMAs once each ----
            nc.tensor.matmul(
                out=warm_ps,
                lhsT=onev[0:1, 0:1],
                rhs=onev[0:1, 0:2],
                start=True,
                stop=True,
            )
            for k in range(KC):
                nc.tensor.matmul(
                    out=warm_ps,
                    lhsT=wtv[0:1, k, 0:1],
                    rhs=wtv[0:1, k, 0:2],
                    start=True,
                    stop=True,
                )
            nc.vector.tensor_copy(out=warm_sb[0:1, 0:1], in_=sgv[0:1, 0:1])
            nc.vector.tensor_copy(out=warm_sb[0:1, 1:2], in_=c16_sb[0:1, 0:1])
            nc.scalar.copy(out=warm_sb[0:1, 2:3], in_=c32_sb[0:1, 0:1])
            nc.scalar.activation(
                out=warm_sb[0:1, 3:4],
                in_=c16_sb[0:1, 0:1],
                func=Copy,
                scale=1.0,
            )

            def emit_readout(pend, lo, hi, after=None):
                xnv, ewt, s = pend
                first = True
                for c in range(lo, hi):
                    ci = s * CPS + c
                    # denominator row first (batched over 4 chunks): its
                    # weight load absorbs the Ew wait so the big matmul only
                    # waits on xn DMA
                    if c % 4 == 0:
                        dmm = nc.tensor.matmul(
                            out=den_acc,
                            lhsT=onev[:, 0:1],
                            rhs=ewt[c // 4],
                            start=(ci == 0),
                            stop=(ci == NCH - 4),
                        )
                        if after is not None and first:
                            tile.add_dep_helper(
                                dmm.ins, after.ins, sync=False,
                                reason="readout batch rides behind its gate",
                            )
                            first = False
                    nc.tensor.matmul(
                        out=out_acc,
                        lhsT=ewt[c // 4][:, (c % 4) * P:(c % 4 + 1) * P],
                        rhs=xnv[:, c, :],
                        start=(ci == 0),
                        stop=(ci == NCH - 1),
                    )

            pend = None  # (xnv, ews, s) of the unreduced superblock
            for s in range(NS):
                xts_t = xtpool.tile([P, KC * SUP], f16, name="xts_t")
                xv = xts_t.rearrange("p (k n) -> p k n", k=KC)
                if s == 0:
                    # split first superblock's load per chunk-pair so the
                    # first gate matmul starts after ~1/8 of the transfer
                    for pr in range(PAIRS):
                        nc.sync.dma_start(
                            out=xv[:, :, pr * 2 * P:(pr + 1) * 2 * P],
                            in_=xts[:, pr * 2 * P:(pr + 1) * 2 * P]
                            .rearrange("(k p) n -> p k n", p=P),
                        )
                else:
                    nc.sync.dma_start(
                        out=xv,
                        in_=xts[:, s * SUP:(s + 1) * SUP]
                        .rearrange("(k p) n -> p k n", p=P),
                    )
                xn_t = xnpool.tile([P, CPS * D], bf16, name="xn_t")
                xnv = xn_t.rearrange("p (c d) -> p c d", c=CPS)
                nc.sync.dma_start(
                    out=xnv,
                    in_=xnb[s * SUP:(s + 1) * SUP, :]
                    .rearrange("(p c) d -> p c d", p=P),
                )

                scol = scpool.tile([P, CPS], f32, name="scol")
                for pair in range(PAIRS):
                    h2 = hpool.tile([P, 2 * D], f32, name="h2", space="PSUM")
                    last_gate = None
                    for half in range(2):
                        c = pair * 2 + half
                        for k in range(KC):
                            last_gate = nc.tensor.matmul(
                                out=h2[:, half * D:(half + 1) * D],
                                lhsT=xv[:, k, c * P:(c + 1) * P],
                                rhs=wtv[:, k, :],
                                start=(k == 0),
                                stop=(k == KC - 1),
                            )
                    if pend is not None:
                        emit_readout(pend, pair * 2, pair * 2 + 2, after=last_gate)
                    g2 = gpool.tile([P, 2 * D], f16, name="g2")
                    nc.scalar.activation(out=g2, in_=h2, func=Tanh)
                    for half in range(2):
                        c = pair * 2 + half
                        junk = jpool.tile([P, D], f16, name="junk")
                        # tensor_tensor_reduce hangs TRN2 hw in this stack;
                        # scalar_tensor_tensor's accum_out does the same
                        # fused multiply + free-dim row-sum
                        nc.vector.scalar_tensor_tensor(
                            out=junk,
                            in0=g2[:, half * D:(half + 1) * D],
                            scalar=1.0,
                            in1=qv,
                            op0=mult,
                            op1=mult,
                            accum_out=scol[:, c:c + 1],
                        )
                ecol = epool.tile([P, CPS], f32, name="ecol")
                # exp in halves so the last superblock's tail pipeline
                # (exp -> ew -> readout) starts as early as possible
                for h in range(2):
                    hcs = CPS // 2
                    nc.scalar.activation(
                        out=ecol[:, h * hcs:(h + 1) * hcs],
                        in_=scol[:, h * hcs:(h + 1) * hcs],
                        func=Exp,
                        bias=shift_sb,
                        scale=1.0,
                    )
                ewt = []
                for c4 in range(CPS // 4):
                    ew = ewpool.tile([P, 4 * P], bf16, name="ew")
                    for j in range(4):
                        c = c4 * 4 + j
                        ci = s * CPS + c
                        nc.vector.tensor_scalar(
                            ew[:, j * P:(j + 1) * P],
                            iov,
                            sgv[:, ci:ci + 1],
                            ecol[:, c:c + 1],
                            is_equal,
                            mult,
                        )
                    ewt.append(ew)
                pend = (xnv, ewt, s)

            emit_readout(pend, 0, CPS)

            ov_sb = opool.tile([P, D], f32, name="ov_sb")
            nc.vector.tensor_copy(out=ov_sb, in_=out_acc)
            od_sb = opool.tile([1, 4 * P], f32, name="od_sb")
            nc.vector.tensor_copy(out=od_sb, in_=den_acc)
            nc.sync.dma_start(out=ov, in_=ov_sb)
            nc.sync.dma_start(out=od, in_=od_sb)

    nc.compile()
    return nc


def _get_module(shard):
    if shard not in _CACHE:
        _CACHE[shard] = build_module(shard)
    return _CACHE[shard]


def pack_consts(W, b, q, nch):
    """Pack the fp16 and f32 constant tensors (seg columns filled per core)."""
    wt = W.T.astype(np.float16)  # [f, fout]
    wt16 = np.ascontiguousarray(
        wt.reshape(KC, P, D).transpose(1, 0, 2).reshape(P, KC * D)
    )
    c16 = np.zeros((P, C16W), dtype=np.float16)
    c16[:, OQ:OQ + D] = q.astype(np.float16)[None, :]
    c16u = c16.view(np.uint16)
    iota_bf = np.arange(P, dtype=np.float32).astype(ml_dtypes.bfloat16)
    c16u[:, OIO:OIO + P] = iota_bf.view(np.uint16)[None, :]
    ones_bf = np.ones(8, dtype=ml_dtypes.bfloat16)
    c16u[:, OONE:OONE + 8] = ones_bf.view(np.uint16)[None, :]

    c32 = np.zeros((P, OSG + nch), dtype=np.float32)
    return wt16, c16, c32


def pack_core(xs, seg, u):
    """Host-side packing of one core's shard -> kernel input dict + glo.

    `u` = solve(W, b): the gate stream ships x + u so the matmul output is
    x @ W.T + b with no on-device bias add; the readout stream keeps x.
    """
    shard = xs.shape[0]
    ns = shard // SUP
    nch = shard // P
    glo = int(seg.min())
    width = int(seg.max()) - glo + 1
    assert width <= P, f"shard graph range {width} > {P} unsupported"
    rel = (seg - glo).astype(np.float32)
    # node (s, 16p + c) lives at chunk c, partition p
    xts = (
        np.ascontiguousarray((xs + u[None, :]).T)
        .reshape(D, ns, P, CPS)
        .swapaxes(2, 3)
        .reshape(D, shard)
        .astype(np.float16)
    )
    xnb = xs.astype(ml_dtypes.bfloat16)
    segc = rel.reshape(ns, P, CPS).transpose(1, 0, 2).reshape(P, nch)
    return {
        "xts": np.ascontiguousarray(xts),
        "xnb": np.ascontiguousarray(xnb),
        "segc": np.ascontiguousarray(segc),
    }, glo


def kernel(**inputs):
    global LAST_RESULT
    from concourse import bass_utils

    x = np.ascontiguousarray(np.asarray(inputs["x"], dtype=np.float32))
    gp = np.asarray(inputs["graph_ptr"]).astype(np.int64)
    W = np.asarray(inputs["W"], dtype=np.float32)
    b = np.asarray(inputs["b"], dtype=np.float32)
    q = np.asarray(inputs["query"], dtype=np.float32)

    N = x.shape[0]
    shard = N // N_CORES
    assert N % N_CORES == 0
    nch = shard // P

    wt16_base, c16_base, c32_base = pack_consts(W, b, q, nch)
    u = np.linalg.solve(W.astype(np.float64), b.astype(np.float64)).astype(
        np.float32
    )

    in_maps = []
    glos = []
    for c in range(N_CORES):
        per, glo = pack_core(
            x[c * shard:(c + 1) * shard], gp[c * shard:(c + 1) * shard], u
        )
        c32 = c32_base.copy()
        c32[:, OSG:OSG + nch] = per.pop("segc")
        per["wt16"] = wt16_base
        per["c16"] = c16_base
        per["c32"] = c32
        in_maps.append(per)
        glos.append(glo)

    nc = _get_module(shard)
    trace = bool(int(os.environ.get("KERNEL_TRACE", "0")))
    res = bass_utils.run_bass_kernel_spmd(
        nc,
        in_maps,
        core_ids=list(range(N_CORES)),
        trace=trace,
        trace_cores=list(range(N_CORES)) if trace else None,
    )
    LAST_RESULT = res

    vec = np.zeros((G, D), dtype=np.float64)
    den = np.zeros((G,), dtype=np.float64)
    for c in range(N_CORES):
        g0 = glos[c]
        g1 = min(G, g0 + P)
        vec[g0:g1] += res.results[c]["ov"][: g1 - g0].astype(np.float64)
        odc = res.results[c]["od"][0].astype(np.float64).reshape(4, P).sum(0)
        den[g0:g1] += odc[: g1 - g0]
    den = np.where(den == 0.0, 1.0, den)
    return (vec / den[:, None]).astype(np.float32)


# revision 10
# speedup vs baseline: 2.1716x; 1.0555x over previous
"""AttnReadout (segment softmax attention readout) Trainium2 kernel, v2.

out[g] = sum_i softmax_within_graph(tanh(x @ W.T + b) @ query)[i] * x[i]

Strategy (8 NeuronCores, data-parallel over nodes, 16384 nodes/core):
  - gate matmul in fp16, node-major PSUM layout:
      H[node, fout] = xts_chunk^T @ W^T        (lhsT = x^T chunk, rhs = W^T)
    fp16 halves the x^T HBM stream and keeps the PE at 1 col/cycle.
  - the tanh bias is folded into the gate on the host:  u = solve(W, b)
    gives (x + u) @ W.T = x @ W.T + b exactly, so xts ships x + u and the
    kernel never adds b (the readout stream xnb keeps the original x).
    u has rms ~0.19 for this problem's W, so fp16 quantization of x + u
    is as accurate as for x.
  - tanh on ScalarE directly from PSUM -> g (fp16)
  - score via DVE scalar_tensor_tensor accum_out:
      score[node] = sum_f g[node,f] * q_rep[node,f]
    This gives scores directly as per-partition columns - no PE matvec and
    no PE transposes (44us of PE time in v1).  (tensor_tensor_reduce hangs
    TRN2 hw in this stack; scalar_tensor_tensor's accum path works.)
  - e = exp(score - 30) on ScalarE, per-superblock [128, 16] batches.
  - Ew[node, g] = (iota[g] == seg_node) * e_node on DVE, output bf16.
    (GpSimd measures ~13x slower per element than DVE here - unusable.)
  - den_row  += ones^T @ Ew                    (PE, bf16, [1, 128])
    out_acc  += Ew^T @ xn_chunk                (PE, bf16 xn, [128, 512])
  - host: accumulate per-graph partials across cores, divide.

Node indexing within a 2048-node superblock is (p, c)-permuted: chunk c,
partition p holds node 16p + c.  This makes each xn DMA descriptor a
contiguous 16KB block (16 consecutive node rows per partition).

exp shift is a global constant so per-core partials add directly; scores
live in [-60, 61] so exp(score - 30) stays in f32/bf16 range and graph-level
softmax accuracy is unaffected.  Measured end-to-end rel err ~2.4e-3
(fp16 gate quantization dominates; tolerance is 2e-2).
"""

import os

import numpy as np
import ml_dtypes

P = 128          # partitions
D = 512          # feature dim
G = 512          # num graphs
N_CORES = 8
SUP = 2048       # nodes per superblock
CPS = SUP // P   # 16 chunks per superblock
PAIRS = CPS // 2
KC = D // P      # 4 contraction chunks
SHIFT = 30.0     # exp(score - SHIFT)

# c16 (fp16 container) column offsets
OQ = 0                   # q replicated to all partitions
OIO = OQ + D             # iota row (bf16 bitcast)
OONE = OIO + P           # ones (bf16 bitcast)
C16W = OONE + 8
# c32 (f32 container) column offsets
OSG = 0                  # per-chunk relative segment ids [128, NCH]

_CACHE = {}
LAST_RESULT = None  # BassKernelResults of the most recent kernel() call


def build_module(shard):
    """Build the Bass/Tile module for one core processing `shard` nodes."""
    import concourse.bacc as bacc
    import concourse.bass as bass  # noqa: F401
    import concourse.mybir as mybir
    import concourse.tile as tile

    f32 = mybir.dt.float32
    f16 = mybir.dt.float16
    bf16 = mybir.dt.bfloat16
    Tanh = mybir.ActivationFunctionType.Tanh
    Exp = mybir.ActivationFunctionType.Exp
    Copy = mybir.ActivationFunctionType.Copy
    is_equal = mybir.AluOpType.is_equal
    mult = mybir.AluOpType.mult
    add = mybir.AluOpType.add

    assert shard % SUP == 0
    NS = shard // SUP            # superblocks
    NCH = shard // P             # total chunks
    C32W = OSG + NCH

    nc = bacc.Bacc("TRN2", target_bir_lowering=False, debug=False, enable_partition_id=False)

    xts = nc.dram_tensor("xts", [D, shard], f16, kind="ExternalInput").ap()
    xnb = nc.dram_tensor("xnb", [shard, D], bf16, kind="ExternalInput").ap()
    wt16 = nc.dram_tensor("wt16", [P, KC * D], f16, kind="ExternalInput").ap()
    c16 = nc.dram_tensor("c16", [P, C16W], f16, kind="ExternalInput").ap()
    c32 = nc.dram_tensor("c32", [P, C32W], f32, kind="ExternalInput").ap()
    ov = nc.dram_tensor("ov", [P, D], f32, kind="ExternalOutput").ap()
    od = nc.dram_tensor("od", [1, 4 * P], f32, kind="ExternalOutput").ap()

    with tile.TileContext(nc) as tc:
        with (
            tc.tile_pool(name="cpool", bufs=1) as cpool,
            tc.tile_pool(name="xtpool", bufs=3) as xtpool,
            tc.tile_pool(name="xnpool", bufs=2) as xnpool,
            tc.tile_pool(name="gpool", bufs=3) as gpool,
            tc.tile_pool(name="jpool", bufs=2) as jpool,
            tc.tile_pool(name="scpool", bufs=2) as scpool,
            tc.tile_pool(name="epool", bufs=2) as epool,
            tc.tile_pool(name="ewpool", bufs=6) as ewpool,
            tc.tile_pool(name="opool", bufs=1) as opool,
            tc.tile_pool(name="hpool", bufs=3, space="PSUM") as hpool,
            tc.tile_pool(name="paccpool", bufs=1, space="PSUM") as paccpool,
        ):
            c16_sb = cpool.tile([P, C16W], f16, name="c16_sb")
            nc.sync.dma_start(out=c16_sb, in_=c16)
            c32_sb = cpool.tile([P, C32W], f32, name="c32_sb")
            nc.sync.dma_start(out=c32_sb, in_=c32)
            wt_sb = cpool.tile([P, KC * D], f16, name="wt_sb")
            wtv = wt_sb.rearrange("p (k m) -> p k m", k=KC)
            for k in range(KC):
                nc.sync.dma_start(
                    out=wtv[:, k, :], in_=wt16[:, k * D:(k + 1) * D]
                )

            qv = c16_sb[:, OQ:OQ + D]
            iov = c16_sb[:, OIO:OIO + P].bitcast(bf16)
            onev = c16_sb[:, OONE:OONE + 8].bitcast(bf16)
            sgv = c32_sb[:, OSG:OSG + NCH]

            shift_sb = cpool.tile([P, 1], f32, name="shift_sb")
            nc.vector.memset(shift_sb, -SHIFT)
            warm_sb = cpool.tile([1, 4], f32, name="warm_sb")

            # ---- long-lived PSUM accumulators ----
            out_acc = paccpool.tile([P, D], f32, name="out_acc", space="PSUM")
            # batched den: 4 chunk-partials side by side, host sums them
            den_acc = paccpool.tile([1, 4 * P], f32, name="den_acc", space="PSUM")
            # warm-up matmuls scribble on den_acc's head; the first real den
            # matmul (start=True) clears the bank before accumulating
            warm_ps = den_acc[0:1, 0:2]

            # ---- engine warm-ups: observe the constant DMAs once each ----
            nc.tensor.matmul(
                out=warm_ps,
                lhsT=onev[0:1, 0:1],
                rhs=onev[0:1, 0:2],
                start=True,
                stop=True,
            )
            for k in range(KC):
                nc.tensor.matmul(
                    out=warm_ps,
                    lhsT=wtv[0:1, k, 0:1],
                    rhs=wtv[0:1, k, 0:2],
                    start=True,
                    stop=True,
                )
            nc.vector.tensor_copy(out=warm_sb[0:1, 0:1], in_=sgv[0:1, 0:1])
            nc.vector.tensor_copy(out=warm_sb[0:1, 1:2], in_=c16_sb[0:1, 0:1])
            nc.scalar.copy(out=warm_sb[0:1, 2:3], in_=c32_sb[0:1, 0:1])
            nc.scalar.activation(
                out=warm_sb[0:1, 3:4],
                in_=c16_sb[0:1, 0:1],
                func=Copy,
                scale=1.0,
            )

            def emit_readout(pend, lo, hi, after=None):
                xnv, ewt, s = pend
                first = True
                for c in range(lo, hi):
                    ci = s * CPS + c
                    # denominator row first (batched over 4 chunks): its
                    # weight load absorbs the Ew wait so the big matmul only
                    # waits on xn DMA
                    if c % 4 == 0:
                        dmm = nc.tensor.matmul(
                            out=den_acc,
                            lhsT=onev[:, 0:1],
                            rhs=ewt[c // 4],
                            start=(ci == 0),
                            stop=(ci == NCH - 4),
                        )
                        if after is not None and first:
                            tile.add_dep_helper(
                                dmm.ins, after.ins, sync=False,
                                reason="readout batch rides behind its gate",
                            )
                            first = False
                    nc.tensor.matmul(
                        out=out_acc,
                        lhsT=ewt[c // 4][:, (c % 4) * P:(c % 4 + 1) * P],
                        rhs=xnv[:, c, :],
                        start=(ci == 0),
                        stop=(ci == NCH - 1),
                    )

            pend = None  # (xnv, ews, s) of the unreduced superblock
            for s in range(NS):
                xts_t = xtpool.tile([P, KC * SUP], f16, name="xts_t")
                xv = xts_t.rearrange("p (k n) -> p k n", k=KC)
                if s == 0:
                    # split first superblock's load per chunk-pair so the
                    # first gate matmul starts after ~1/8 of the transfer
                    for pr in range(PAIRS):
                        nc.sync.dma_start(
                            out=xv[:, :, pr * 2 * P:(pr + 1) * 2 * P],
                            in_=xts[:, pr * 2 * P:(pr + 1) * 2 * P]
                            .rearrange("(k p) n -> p k n", p=P),
                        )
                else:
                    nc.sync.dma_start(
                        out=xv,
                        in_=xts[:, s * SUP:(s + 1) * SUP]
                        .rearrange("(k p) n -> p k n", p=P),
                    )
                xn_t = xnpool.tile([P, CPS * D], bf16, name="xn_t")
                xnv = xn_t.rearrange("p (c d) -> p c d", c=CPS)
                nc.sync.dma_start(
                    out=xnv,
                    in_=xnb[s * SUP:(s + 1) * SUP, :]
                    .rearrange("(p c) d -> p c d", p=P),
                )

                scol = scpool.tile([P, CPS], f32, name="scol")
                for pair in range(PAIRS):
                    h2 = hpool.tile([P, 2 * D], f32, name="h2", space="PSUM")
                    last_gate = None
                    for half in range(2):
                        c = pair * 2 + half
                        for k in range(KC):
                            last_gate = nc.tensor.matmul(
                                out=h2[:, half * D:(half + 1) * D],
                                lhsT=xv[:, k, c * P:(c + 1) * P],
                                rhs=wtv[:, k, :],
                                start=(k == 0),
                                stop=(k == KC - 1),
                            )
                    if pend is not None:
                        emit_readout(pend, pair * 2, pair * 2 + 2, after=last_gate)
                    g2 = gpool.tile([P, 2 * D], f16, name="g2")
                    nc.scalar.activation(out=g2, in_=h2, func=Tanh)
                    for half in range(2):
                        c = pair * 2 + half
                        junk = jpool.tile([P, D], f16, name="junk")
                        # tensor_tensor_reduce hangs TRN2 hw in this stack;
                        # scalar_tensor_tensor's accum_out does the same
                        # fused multiply + free-dim row-sum
                        nc.vector.scalar_tensor_tensor(
                            out=junk,
                            in0=g2[:, half * D:(half + 1) * D],
                            scalar=1.0,
                            in1=qv,
                            op0=mult,
                            op1=mult,
                            accum_out=scol[:, c:c + 1],
                        )
                ecol = epool.tile([P, CPS], f32, name="ecol")
                # exp in halves so the last superblock's tail pipeline
                # (exp -> ew -> readout) starts as early as possible
                for h in range(2):
                    hcs = CPS // 2
                    nc.scalar.activation(
                        out=ecol[:, h * hcs:(h + 1) * hcs],
                        in_=scol[:, h * hcs:(h + 1) * hcs],
                        func=Exp,
                        bias=shift_sb,
                        scale=1.0,
                    )
                ewt = []
                for c4 in range(CPS // 4):
                    ew = ewpool.tile([P, 4 * P], bf16, name="ew")
                    for j in range(4):
                        c = c4 * 4 + j
                        ci = s * CPS + c
                        nc.vector.tensor_scalar(
                            ew[:, j * P:(j + 1) * P],
                            iov,
                            sgv[:, ci:ci + 1],
                            ecol[:, c:c + 1],
                            is_equal,
                            mult,
                        )
                    ewt.append(ew)
                pend = (xnv, ewt, s)

            emit_readout(pend, 0, CPS)

            ov_sb = opool.tile([P, D], f32, name="ov_sb")
            nc.vector.tensor_copy(out=ov_sb, in_=out_acc)
            od_sb = opool.tile([1, 4 * P], f32, name="od_sb")
            nc.vector.tensor_copy(out=od_sb, in_=den_acc)
            nc.sync.dma_start(out=ov, in_=ov_sb)
            nc.sync.dma_start(out=od, in_=od_sb)

    nc.compile()
    return nc


def _get_module(shard):
    if shard not in _CACHE:
        _CACHE[shard] = build_module(shard)
    return _CACHE[shard]


def pack_consts(W, b, q, nch):
    """Pack the fp16 and f32 constant tensors (seg columns filled per core)."""
    wt = W.T.astype(np.float16)  # [f, fout]
    wt16 = np.ascontiguousarray(
        wt.reshape(KC, P, D).transpose(1, 0, 2).reshape(P, KC * D)
    )
    c16 = np.zeros((P, C16W), dtype=np.float16)
    c16[:, OQ:OQ + D] = q.astype(np.float16)[None, :]
    c16u = c16.view(np.uint16)
    iota_bf = np.arange(P, dtype=np.float32).astype(ml_dtypes.bfloat16)
    c16u[:, OIO:OIO + P] = iota_bf.view(np.uint16)[None, :]
    ones_bf = np.ones(8, dtype=ml_dtypes.bfloat16)
    c16u[:, OONE:OONE + 8] = ones_bf.view(np.uint16)[None, :]

    c32 = np.zeros((P, OSG + nch), dtype=np.float32)
    return wt16, c16, c32


def pack_core(xs, seg, u):
    """Host-side packing of one core's shard -> kernel input dict + glo.

    `u` = solve(W, b): the gate stream ships x + u so the matmul output is
    x @ W.T + b with no on-device bias add; the readout stream keeps x.
    """
    shard = xs.shape[0]
    ns = shard // SUP
    nch = shard // P
    glo = int(seg.min())
    width = int(seg.max()) - glo + 1
    assert width <= P, f"shard graph range {width} > {P} unsupported"
    rel = (seg - glo).astype(np.float32)
    # node (s, 16p + c) lives at chunk c, partition p
    xts = (
        np.ascontiguousarray((xs + u[None, :]).T)
        .reshape(D, ns, P, CPS)
        .swapaxes(2, 3)
        .reshape(D, shard)
        .astype(np.float16)
    )
    xnb = xs.astype(ml_dtypes.bfloat16)
    segc = rel.reshape(ns, P, CPS).transpose(1, 0, 2).reshape(P, nch)
    return {
        "xts": np.ascontiguousarray(xts),
        "xnb": np.ascontiguousarray(xnb),
        "segc": np.ascontiguousarray(segc),
    }, glo


def kernel(**inputs):
    global LAST_RESULT
    from concourse import bass_utils

    x = np.ascontiguousarray(np.asarray(inputs["x"], dtype=np.float32))
    gp = np.asarray(inputs["graph_ptr"]).astype(np.int64)
    W = np.asarray(inputs["W"], dtype=np.float32)
    b = np.asarray(inputs["b"], dtype=np.float32)
    q = np.asarray(inputs["query"], dtype=np.float32)

    N = x.shape[0]
    shard = N // N_CORES
    assert N % N_CORES == 0
    nch = shard // P

    wt16_base, c16_base, c32_base = pack_consts(W, b, q, nch)
    u = np.linalg.solve(W.astype(np.float64), b.astype(np.float64)).astype(
        np.float32
    )

    in_maps = []
    glos = []
    for c in range(N_CORES):
        per, glo = pack_core(
            x[c * shard:(c + 1) * shard], gp[c * shard:(c + 1) * shard], u
        )
        c32 = c32_base.copy()
        c32[:, OSG:OSG + nch] = per.pop("segc")
        per["wt16"] = wt16_base
        per["c16"] = c16_base
        per["c32"] = c32
        in_maps.append(per)
        glos.append(glo)

    nc = _get_module(shard)
    trace = bool(int(os.environ.get("KERNEL_TRACE", "0")))
    res = bass_utils.run_bass_kernel_spmd(
        nc,
        in_maps,
        core_ids=list(range(N_CORES)),
        trace=trace,
        trace_cores=list(range(N_CORES)) if trace else None,
    )
    LAST_RESULT = res

    vec = np.zeros((G, D), dtype=np.float64)
    den = np.zeros((G,), dtype=np.float64)
    for c in range(N_CORES):
        g0 = glos[c]
        g1 = min(G, g0 + P)
        vec[g0:g1] += res.results[c]["ov"][: g1 - g0].astype(np.float64)
        odc = res.results[c]["od"][0].astype(np.float64).reshape(4, P).sum(0)
        den[g0:g1] += odc[: g1 - g0]
    den = np.where(den == 0.0, 1.0, den)
    return (vec / den[:, None]).astype(np.float32)
